# revision 11
# baseline (speedup 1.0000x reference)
"""Trainium2 Bass kernel for DequantingLinear (GGML Q8_0 dequant + linear).

Computes out[4096, 12288] = x[4096, 3072] @ dequant(w_q, w_scales).T + bias
where w_q is int32 (int8-valued) with per-32-element-block fp32 scales.

Sharding: tensor-parallel over output features across 8 NeuronCores. Each
core gets the full x and a 1536-row shard of w_q / w_scales / bias,
computes its [4096, 1536] output slice; the host concatenates on axis 1.

Per-core kernel (Tile framework), v2 — no DRAM bounces, LDW-deduped GEMM:

  * x path: per 128-token group, one SWDGE cast DMA moves x rows straight
    from DRAM fp32 into SBUF bf16 [128, 3072], then ONE SBUF->SBUF xbar
    transpose lands the k-major tile xt_g [128, 24, 128] (contiguous xbar
    destination - non-contiguous dests corrupt on HW).  No bf16 bounce
    ring in DRAM; x is read from HBM exactly once.
  * w path: per 128-row o-tile, HWDGE load of w_q int32, one mixed-dtype
    vector multiply (int32 x block-broadcast fp32 scales -> bf16, exact
    for |q|<=127), then ONE SBUF->SBUF xbar transpose into the resident
    wt [128, OT=12, KT=24, 128] (o-major so each dest is contiguous).
  * GEMM: for each token group, k-outer / n-inner so the three matmuls
    sharing the stationary xt_g[:, k, :] run back-to-back; a post-build
    BIR pass (dedup_ldweights) removes the redundant LDWEIGHTS bacc emits
    per matmul, so the stationary is loaded once per k instead of three
    times.  The moving operand for (k, n) is the 3D AP
    wt[:, 4n:4n+4, k, :] (512 output columns).  psum [128, 512] fp32
    accumulates 24 k-matmuls; bias is added during the PSUM->SBUF drain
    on the vector engine.
  * phase-1: the first two token blocks' n=0 GEMM groups are emitted
    early so the PE ramps while the remaining w o-tiles stream.
  All HWDGE DMAs are issued on nc.sync - ACT-issued DMAs were observed to
  produce corrupted results on hardware in this configuration.
"""

import sys

for _p in ("/opt/trn_rl_repo",):
    if _p not in sys.path:
        sys.path.append(_p)

from contextlib import ExitStack

import numpy as np

import concourse.bacc as bacc
import concourse.bass as bass
import concourse.mybir as mybir
from concourse import tile
from concourse.bass_utils import run_bass_kernel_spmd

FP32 = mybir.dt.float32
BF16 = mybir.dt.bfloat16
INT32 = mybir.dt.int32

N_CORES = 8
TOK, IN, OUT = 4096, 3072, 12288
QK = 32
OUT_SH = OUT // N_CORES
TOK_BLK = 512
NCOL = 512
NB1 = 2


def _ap_key(ap):
    return (
        ap.memref,
        ap.offset,
        str(ap.ap),
        str(ap.dtype),
        str(getattr(ap, "kind", "")),
    )


def dedup_ldweights(nc):
    """Remove redundant InstLdweights: the PE array keeps the stationary
    operand loaded across matmuls, so consecutive PE-stream Ldweights with
    identical weight APs are redundant.  Waits of a removed Ldweights merge
    into its paired Matmult (same engine stream; semantics preserved)."""
    removed = 0
    for f in nc.m.functions:
        for blk in f.blocks:
            insts = blk.instructions
            prev_key = None
            drop = []
            for i, inst in enumerate(insts):
                if isinstance(inst, mybir.InstLdweights):
                    key = _ap_key(inst.ins[0])
                    nxt = insts[i + 1] if i + 1 < len(insts) else None
                    si = inst.sync_info
                    has_update = bool(si and si.on_update)
                    if (
                        key == prev_key
                        and not has_update
                        and isinstance(nxt, mybir.InstMatmult)
                    ):
                        if si and si.on_wait:
                            nsi = nxt.sync_info
                            if nsi is None:
                                nxt.sync_info = mybir.SyncInfo(
                                    on_wait=list(si.on_wait), on_update=[]
                                )
                            else:
                                nsi.on_wait = list(nsi.on_wait) + list(si.on_wait)
                        drop.append(i)
                        removed += 1
                    else:
                        prev_key = key
                elif isinstance(inst, mybir.InstMatmult):
                    pass  # matmul does not disturb loaded weights
                elif getattr(inst, "engine", None) == mybir.EngineType.PE:
                    prev_key = None
            for i in reversed(drop):
                del insts[i]
    return removed


def _build(nc: bass.Bass, repeats: int = 1):
    P = 128
    KT = IN // P           # 24
    NBLK = TOK // TOK_BLK  # 8
    MT = TOK_BLK // P      # 4
    NT = OUT_SH // NCOL    # 3
    NB = IN // QK          # 96
    OT = OUT_SH // P       # 12
    OPN = NCOL // P        # 4

    x = nc.dram_tensor("x", [TOK, IN], FP32, kind="ExternalInput")
    w_q = nc.dram_tensor("w_q", [OUT_SH, IN], INT32, kind="ExternalInput")
    w_scales = nc.dram_tensor("w_scales", [OUT_SH, NB], FP32, kind="ExternalInput")
    bias = nc.dram_tensor("bias", [OUT_SH], FP32, kind="ExternalInput")
    outs_t = [
        nc.dram_tensor(f"out{r}" if r else "out", [TOK, OUT_SH], FP32,
                       kind="ExternalOutput")
        for r in range(repeats)
    ]

    with tile.TileContext(nc) as tc, ExitStack() as ctx:
        const_pool = ctx.enter_context(tc.tile_pool(name="const", bufs=1))
        wq_pool = ctx.enter_context(tc.tile_pool(name="wq", bufs=3))
        wd_pool = ctx.enter_context(tc.tile_pool(name="wd", bufs=2))
        wt_pool = ctx.enter_context(tc.tile_pool(name="wt", bufs=1))
        xb_pool = ctx.enter_context(tc.tile_pool(name="xb", bufs=3))
        xt_pool = ctx.enter_context(tc.tile_pool(name="xt", bufs=8))
        out_pool = ctx.enter_context(tc.tile_pool(name="out", bufs=4))
        psum_pool = ctx.enter_context(tc.tile_pool(name="psum", bufs=8, space="PSUM"))

        bias_rep = const_pool.tile([P, OUT_SH], FP32, tag="bias_rep")
        nc.sync.dma_start(bias_rep[:], bias.ap().unsqueeze(0).to_broadcast([P, OUT_SH]))

        for rep in range(repeats):
            out = outs_t[rep]

            def load_xt_group(g):
                # the 3-slot xb pool naturally paces x casts; no explicit
                # pacing dep needed (and one would over-serialize pass chains)
                xb = xb_pool.tile([P, IN], BF16, tag="xb", name=f"xb_{rep}_{g}")
                nc.gpsimd.dma_start(xb[:], x.ap()[g * P : (g + 1) * P, :])
                xt_g = xt_pool.tile([P, KT, P], BF16, tag="xt", name=f"xt_{rep}_{g}")
                nc.sync.dma_start(xt_g[:, :, :], xb[:], transpose=True)
                return xt_g

            sc_tiles = []
            for o in range(OT):
                sct = const_pool.tile([P, NB], FP32, tag=f"sc_{o}",
                                      name=f"sct_{rep}_{o}")
                nc.sync.dma_start(sct[:], w_scales.ap()[o * P : (o + 1) * P, :])
                sc_tiles.append(sct)

            # head: interleave w o-tiles with x groups so the w chain's
            # HWDGE DMAs aren't queued behind all eight x transposes
            xt_cache = {}
            wt = wt_pool.tile([P, OT, KT, P], BF16, tag="wt", name=f"wt_{rep}")
            for o in range(OT):
                rows = slice(o * P, (o + 1) * P)
                wq_i = wq_pool.tile([P, IN], INT32, tag="wq", name=f"wq_{rep}_{o}")
                nc.sync.dma_start(wq_i[:], w_q.ap()[rows, :])
                wd = wd_pool.tile([P, IN], BF16, tag="wd", name=f"wd_{rep}_{o}")
                nc.vector.tensor_mul(
                    wd[:].rearrange("p (b q) -> p b q", q=QK),
                    wq_i[:].rearrange("p (b q) -> p b q", q=QK),
                    sc_tiles[o][:].unsqueeze(2).to_broadcast([P, NB, QK]),
                )
                nc.sync.dma_start(wt[:, o, :, :], wd[:], transpose=True)
                if o < NB1 * MT:
                    xt_cache[o] = load_xt_group(o)

            def gemm_group(xt_g, g, ns):
                tok0 = g * P
                pss = [
                    psum_pool.tile([P, NCOL], FP32, tag="ps",
                                   name=f"ps_{rep}_{g}_{n}")
                    for n in ns
                ]
                for k in range(KT):
                    for i, n in enumerate(ns):
                        nc.tensor.matmul(
                            pss[i][:],
                            xt_g[:, k, :],
                            wt[:, n * OPN : (n + 1) * OPN, k, :],
                            start=(k == 0),
                            stop=(k == KT - 1),
                        )
                for i, n in enumerate(ns):
                    ob = out_pool.tile([P, NCOL], FP32, tag="ob",
                                       name=f"ob_{rep}_{g}_{n}")
                    nc.vector.tensor_add(
                        ob[:], pss[i][:], bias_rep[:, n * NCOL : (n + 1) * NCOL]
                    )
                    nc.sync.dma_start(
                        out.ap()[tok0 : tok0 + P, n * NCOL : (n + 1) * NCOL], ob[:]
                    )

            for b in range(NB1):
                for gg in range(MT):
                    gemm_group(xt_cache[b * MT + gg], b * MT + gg, [0])

            for b in range(NBLK):
                for gg in range(MT):
                    g = b * MT + gg
                    if g in xt_cache:
                        xt_g = xt_cache.pop(g)
                    else:
                        xt_g = load_xt_group(g)
                    gemm_group(xt_g, g, [1, 2] if b < NB1 else [0, 1, 2])
    return nc


_COMPILED_NC = None


def _get_nc():
    global _COMPILED_NC
    if _COMPILED_NC is None:
        nc = bacc.Bacc("TRN2", target_bir_lowering=False, debug=False)
        _build(nc)
        dedup_ldweights(nc)
        nc.compile()
        _COMPILED_NC = nc
    return _COMPILED_NC


def kernel(x, w_q, w_scales, bias):
    assert x.shape == (TOK, IN) and w_q.shape == (OUT, IN)
    nc = _get_nc()
    x = np.ascontiguousarray(np.asarray(x, dtype=np.float32))
    w_q = np.asarray(w_q, dtype=np.int32)
    w_scales = np.asarray(w_scales, dtype=np.float32)
    bias = np.asarray(bias, dtype=np.float32)
    in_maps = []
    for c in range(N_CORES):
        r = slice(c * OUT_SH, (c + 1) * OUT_SH)
        in_maps.append(
            {
                "x": x,
                "w_q": np.ascontiguousarray(w_q[r]),
                "w_scales": np.ascontiguousarray(w_scales[r]),
                "bias": np.ascontiguousarray(bias[r]),
            }
        )
    res = run_bass_kernel_spmd(nc, in_maps, list(range(N_CORES)))
    return np.concatenate([res.results[c]["out"] for c in range(N_CORES)], axis=1)
